# revision 7
# baseline (speedup 1.0000x reference)
"""Trainium2 Bass kernel for nn_AddDropMRR (add-drop microring resonator).

Math: for real inputs x (input_signal) and a (add_signal), the reference's
complex pipeline reduces to two magnitude outputs expressible with
per-wavelength REAL coefficient vectors (prologue-computed on device):

  e1 = x^2, e2 = (s*a)^2, e3 = x*a
  R2 = |ring_with_add|^2 = m1*e1 + m2*e3 + e2
  W2 = |through|^2       = n1*e1 + n2*e3 + n3*R2
  through = sqrt(W2);  drop = sqrt(k2c^2 * R2)

Sharding: wavelength dim (8192) split across 8 cores (1024 each). Shards are
host-transposed so wavelength lies on SBUF partitions; per-wavelength
coefficients become [128,1] per-partition scalars feeding fused
scalar_tensor_tensor ops on the vector engine, squares/sqrts on the scalar
engine. All heavy traffic (2 in + 2 out tiles of [128, 2048] f32 per chunk)
is contiguous 1 MiB DMA.
"""
import numpy as np

B = 2048          # batch
W = 8192          # wavelengths
NCORES = 8
WSH = W // NCORES  # 1024 wavelengths per core
P = 128            # SBUF partitions
NCHUNK = WSH // P  # 8 chunks per core
N_EFF = 2.4
CIRC = 2.0 * np.pi * 1e-05
KC = 2.0 * np.pi * N_EFF * CIRC
TWO_PI = float(2.0 * np.pi)


def _host_scalars(coupling_1, coupling_2, phi_1, phi_2, phi_ring, alpha):
    c1 = float(np.asarray(coupling_1).reshape(-1)[0])
    c2 = float(np.asarray(coupling_2).reshape(-1)[0])
    p1 = float(np.asarray(phi_1).reshape(-1)[0])
    pr = float(np.asarray(phi_ring).reshape(-1)[0])
    al = float(np.asarray(alpha).reshape(-1)[0])
    k1c = float(np.clip(c1, 0.01, 0.99))
    k2c = float(np.clip(c2, 0.01, 0.99))
    t1 = float(np.sqrt(1.0 - k1c * k1c))
    t2 = float(np.sqrt(1.0 - k2c * k2c))
    s = float(np.sqrt(c2))      # unclamped, as in reference
    s1 = float(np.sqrt(c1))     # unclamped
    kappa = float(al * np.sqrt(1.0 - c1 * c1) * np.sqrt(1.0 - c2 * c2))  # unclamped
    return dict(
        k1c=k1c, k2c=k2c, t1=t1, t2=t2, s=s, s1=s1, kappa=kappa, alpha=al,
        phi1=p1, phiring=pr,
        m1=(k1c * al) ** 2,
        t2s1=t2 * s1,
    )


def _build_graph(sc, loop_n=1, nchunk=NCHUNK, main_bufs=2, tmp_bufs=3, mode="full"):
    """Build the SPMD per-core graph. loop_n>1 wraps everything in a For_i
    hardware loop (identical body each iteration) for steady-state timing.
    mode: full | dma | act | dve | prologue (non-full modes are for
    engine-isolation timing probes only; outputs are garbage)."""
    import concourse.tile as tile
    from concourse import bacc, mybir

    f32 = mybir.dt.float32
    i32 = mybir.dt.int32
    AF = mybir.ActivationFunctionType
    ALU = mybir.AluOpType

    wsh = nchunk * P
    nc = bacc.Bacc("TRN2", target_bir_lowering=False, debug=False,
                   num_devices=NCORES)
    x_ext = nc.declare_dram_parameter("x_t", [wsh, B], f32, isOutput=False)
    a_ext = nc.declare_dram_parameter("a_t", [wsh, B], f32, isOutput=False)
    wl_ext = nc.declare_dram_parameter("wl_t", [P, nchunk], f32, isOutput=False)
    o1_ext = nc.declare_dram_parameter("o1_t", [wsh, B], f32, isOutput=True)
    o2_ext = nc.declare_dram_parameter("o2_t", [wsh, B], f32, isOutput=True)

    with tile.TileContext(nc) as tc:
        with tc.tile_pool(name="cst", bufs=1) as cst, \
             tc.tile_pool(name="mio", bufs=main_bufs) as mio, \
             tc.tile_pool(name="mtmp", bufs=tmp_bufs) as mtmp:

            def body(_iv=None):
                # ---------------- prologue: per-wavelength coefficients -----
                _tag = [0]

                def t(shape=(P, nchunk), dt=f32):
                    _tag[0] += 1
                    return cst.tile(list(shape), dt, tag=f"cst{_tag[0]}", name=f"cst{_tag[0]}")

                wlt = t()
                nc.sync.dma_start(wlt[:], wl_ext[:])
                inv = t()
                nc.vector.reciprocal(inv[:], wlt[:])

                # two range-reduced angles: phi and phi + pi/2
                trig = []
                for bias in (sc["phiring"], sc["phiring"] + np.pi / 2):
                    ang = t()
                    nc.vector.tensor_scalar(ang[:], inv[:], KC, float(bias),
                                            ALU.mult, ALU.add)
                    u = t()
                    nc.vector.tensor_scalar(u[:], ang[:], 1.0 / TWO_PI, None,
                                            ALU.mult)
                    ki = t(dt=i32)
                    nc.vector.tensor_copy(ki[:], u[:])
                    kf = t()
                    nc.vector.tensor_copy(kf[:], ki[:])
                    angm = t()
                    nc.vector.scalar_tensor_tensor(
                        out=angm[:], in0=kf[:], scalar=-TWO_PI, in1=ang[:],
                        op0=ALU.mult, op1=ALU.add)
                    sn = t()
                    nc.scalar.activation(sn[:], angm[:], AF.Sin)
                    trig.append(sn)
                sin_phi, cos_phi = trig

                # rotate by phi1 using host sin/cos (phi1 is a scalar input)
                c1h = float(np.cos(sc["phi1"]))
                s1h = float(np.sin(sc["phi1"]))
                tmp = t()
                nc.vector.tensor_scalar(tmp[:], cos_phi[:], s1h, None, ALU.mult)
                sin_p1 = t()
                nc.vector.scalar_tensor_tensor(
                    out=sin_p1[:], in0=sin_phi[:], scalar=c1h, in1=tmp[:],
                    op0=ALU.mult, op1=ALU.add)
                tmp2 = t()
                nc.vector.tensor_scalar(tmp2[:], sin_phi[:], s1h, None, ALU.mult)
                cos_p1 = t()
                nc.vector.scalar_tensor_tensor(
                    out=cos_p1[:], in0=cos_phi[:], scalar=c1h, in1=tmp2[:],
                    op0=ALU.mult, op1=ALU.subtract)

                ka = sc["k1c"] * sc["alpha"]
                Pv = t()
                nc.vector.tensor_scalar(Pv[:], sin_p1[:], -ka, None, ALU.mult)
                Qv = t()
                nc.vector.tensor_scalar(Qv[:], cos_p1[:], ka, None, ALU.mult)

                den_re = t()
                nc.vector.tensor_scalar(den_re[:], cos_phi[:], -sc["kappa"], 1.0,
                                        ALU.mult, ALU.add)
                d2 = t()
                nc.vector.tensor_mul(d2[:], den_re[:], den_re[:])
                s2q = t()
                nc.vector.tensor_mul(s2q[:], sin_phi[:], sin_phi[:])
                den2 = t()
                nc.vector.scalar_tensor_tensor(
                    out=den2[:], in0=s2q[:], scalar=sc["kappa"] ** 2, in1=d2[:],
                    op0=ALU.mult, op1=ALU.add)
                invd = t()
                nc.vector.reciprocal(invd[:], den2[:])

                g0 = t()
                nc.vector.tensor_mul(g0[:], den_re[:], invd[:])
                Gre = t()
                nc.vector.tensor_scalar(Gre[:], g0[:], sc["t2s1"], None, ALU.mult)
                g1 = t()
                nc.vector.tensor_mul(g1[:], sin_phi[:], invd[:])
                Gim = t()
                nc.vector.tensor_scalar(Gim[:], g1[:], sc["t2s1"] * sc["kappa"],
                                        None, ALU.mult)

                m2v = t()
                nc.vector.tensor_scalar(m2v[:], Pv[:], 2.0 * sc["s"], None, ALU.mult)
                n3v = t()
                nc.vector.tensor_scalar(n3v[:], invd[:], sc["t2s1"] ** 2, None,
                                        ALU.mult)
                z1 = t()
                nc.vector.tensor_mul(z1[:], Gre[:], Pv[:])
                z2 = t()
                nc.vector.tensor_mul(z2[:], Gim[:], Qv[:])
                z3 = t()
                nc.vector.tensor_sub(z3[:], z1[:], z2[:])
                n1v = t()
                nc.vector.tensor_scalar(n1v[:], z3[:], 2.0 * sc["t1"],
                                        sc["t1"] ** 2, ALU.mult, ALU.add)
                n2v = t()
                nc.vector.tensor_scalar(n2v[:], Gre[:], 2.0 * sc["t1"] * sc["s"],
                                        None, ALU.mult)

                # ---------------- main loop over wavelength chunks ----------
                if mode == "prologue":
                    # touch vectors so they aren't dead: copy to o1 corner
                    dummy = mio.tile([P, nchunk], f32, tag="dummy", name="dummy")
                    nc.vector.tensor_add(dummy[:], m2v[:], n1v[:])
                    nc.sync.dma_start(o1_ext[0:P, 0:nchunk], dummy[:])
                    return
                if mode == "full":
                    rows = 4
                    nblk = nchunk // rows
                    FB = B * rows
                    xv = x_ext.ap().rearrange("(n p r) b -> n p (r b)", p=P, r=rows)
                    av = a_ext.ap().rearrange("(n p r) b -> n p (r b)", p=P, r=rows)
                    o1v = o1_ext.ap().rearrange("(n p r) b -> n p (r b)", p=P, r=rows)
                    o2v = o2_ext.ap().rearrange("(n p r) b -> n p (r b)", p=P, r=rows)
                    for n in range(nblk):
                        xt = mio.tile([P, FB], f32, tag="xt", name="xt")
                        nc.sync.dma_start(xt[:], xv[n])
                        at = mio.tile([P, FB], f32, tag="at", name="at")
                        nc.sync.dma_start(at[:], av[n])
                        for q in range(rows):
                            qs = slice(q * B, (q + 1) * B)
                            ci = n * rows + q
                            e1 = mtmp.tile([P, B], f32, tag="e1", name="e1")
                            nc.scalar.activation(e1[:], xt[:, qs], AF.Square)
                            e3 = mtmp.tile([P, B], f32, tag="e3", name="e3")
                            nc.vector.tensor_mul(e3[:], xt[:, qs], at[:, qs])
                            # square a in place (after e3 consumed raw a)
                            nc.scalar.activation(at[:, qs], at[:, qs], AF.Square,
                                                 scale=sc["s"])
                            # R2 accumulates in place into at quarter
                            nc.vector.scalar_tensor_tensor(
                                out=at[:, qs], in0=e1[:], scalar=sc["m1"],
                                in1=at[:, qs], op0=ALU.mult, op1=ALU.add)
                            nc.vector.scalar_tensor_tensor(
                                out=at[:, qs], in0=e3[:],
                                scalar=m2v[:, ci:ci + 1], in1=at[:, qs],
                                op0=ALU.mult, op1=ALU.add)
                            # W2 accumulates in place into xt quarter
                            nc.scalar.activation(xt[:, qs], e1[:], AF.Copy,
                                                 scale=n1v[:, ci:ci + 1])
                            nc.vector.scalar_tensor_tensor(
                                out=xt[:, qs], in0=e3[:],
                                scalar=n2v[:, ci:ci + 1], in1=xt[:, qs],
                                op0=ALU.mult, op1=ALU.add)
                            nc.vector.scalar_tensor_tensor(
                                out=xt[:, qs], in0=at[:, qs],
                                scalar=n3v[:, ci:ci + 1], in1=xt[:, qs],
                                op0=ALU.mult, op1=ALU.add)
                            # in-place sqrts (R2 consumed above before this)
                            nc.scalar.activation(at[:, qs], at[:, qs], AF.Sqrt,
                                                 scale=sc["k2c"] ** 2)
                            nc.scalar.activation(xt[:, qs], xt[:, qs], AF.Sqrt)
                        nc.sync.dma_start(o1v[n], xt[:])
                        nc.sync.dma_start(o2v[n], at[:])
                    return
                for c in range(nchunk):
                    rs = slice(c * P, (c + 1) * P)
                    xt = mio.tile([P, B], f32, tag="xt", name="xt")
                    nc.sync.dma_start(xt[:], x_ext[rs, :])
                    at = mio.tile([P, B], f32, tag="at", name="at")
                    nc.sync.dma_start(at[:], a_ext[rs, :])
                    if mode == "dma":
                        nc.sync.dma_start(o1_ext[rs, :], xt[:])
                        nc.sync.dma_start(o2_ext[rs, :], at[:])
                        continue
                    if mode == "act":
                        e1 = mtmp.tile([P, B], f32, tag="e1", name="e1")
                        nc.scalar.activation(e1[:], xt[:], AF.Square)
                        e2 = mtmp.tile([P, B], f32, tag="e2", name="e2")
                        nc.scalar.activation(e2[:], at[:], AF.Square, scale=sc["s"])
                        o2t = mio.tile([P, B], f32, tag="o2t", name="o2t")
                        nc.scalar.activation(o2t[:], e2[:], AF.Sqrt,
                                             scale=sc["k2c"] ** 2)
                        w0 = mtmp.tile([P, B], f32, tag="w0", name="w0")
                        nc.scalar.activation(w0[:], e1[:], AF.Copy,
                                             scale=n1v[:, c:c + 1])
                        o1t = mio.tile([P, B], f32, tag="o1t", name="o1t")
                        nc.scalar.activation(o1t[:], w0[:], AF.Sqrt)
                        nc.sync.dma_start(o1_ext[rs, :], o1t[:])
                        nc.sync.dma_start(o2_ext[rs, :], o2t[:])
                        continue
                    if mode == "dve":
                        e3 = mtmp.tile([P, B], f32, tag="e3", name="e3")
                        nc.vector.tensor_mul(e3[:], xt[:], at[:])
                        e2 = mtmp.tile([P, B], f32, tag="e2", name="e2")
                        nc.vector.scalar_tensor_tensor(
                            out=e2[:], in0=e3[:], scalar=sc["m1"], in1=xt[:],
                            op0=ALU.mult, op1=ALU.add)
                        nc.vector.scalar_tensor_tensor(
                            out=e2[:], in0=e3[:], scalar=m2v[:, c:c + 1], in1=e2[:],
                            op0=ALU.mult, op1=ALU.add)
                        w0 = mtmp.tile([P, B], f32, tag="w0", name="w0")
                        nc.vector.scalar_tensor_tensor(
                            out=w0[:], in0=e3[:], scalar=n2v[:, c:c + 1], in1=e2[:],
                            op0=ALU.mult, op1=ALU.add)
                        nc.vector.scalar_tensor_tensor(
                            out=w0[:], in0=e2[:], scalar=n3v[:, c:c + 1], in1=w0[:],
                            op0=ALU.mult, op1=ALU.add)
                        nc.sync.dma_start(o1_ext[rs, :], w0[:])
                        nc.sync.dma_start(o2_ext[rs, :], e2[:])
                        continue


            if loop_n > 1:
                with tc.For_i(0, loop_n, 1):
                    body()
            else:
                body()

    nc.compile()
    return nc


def _shard_inputs(input_signal, add_signal, wavelengths):
    x = np.ascontiguousarray(np.asarray(input_signal, dtype=np.float32))
    a = np.ascontiguousarray(np.asarray(add_signal, dtype=np.float32))
    wl = np.ascontiguousarray(np.asarray(wavelengths, dtype=np.float32))
    in_maps = []
    for i in range(NCORES):
        sl = slice(i * WSH, (i + 1) * WSH)
        in_maps.append({
            "x_t": np.ascontiguousarray(x[:, sl].T),
            "a_t": np.ascontiguousarray(a[:, sl].T),
            "wl_t": np.ascontiguousarray(
                wl[sl].reshape(NCHUNK // 4, P, 4).transpose(1, 0, 2).reshape(P, NCHUNK)),
        })
    return in_maps


def _gather_outputs(results):
    through = np.empty((B, W), np.float32)
    drop = np.empty((B, W), np.float32)
    for i in range(NCORES):
        sl = slice(i * WSH, (i + 1) * WSH)
        through[:, sl] = results[i]["o1_t"].T
        drop[:, sl] = results[i]["o2_t"].T
    return through, drop


def kernel(input_signal, add_signal, wavelengths, coupling_1, coupling_2,
           phi_1, phi_2, phi_ring, alpha):
    from concourse.bass_utils import run_bass_kernel_spmd

    sc = _host_scalars(coupling_1, coupling_2, phi_1, phi_2, phi_ring, alpha)
    nc = _build_graph(sc)
    in_maps = _shard_inputs(input_signal, add_signal, wavelengths)
    res = run_bass_kernel_spmd(nc, in_maps, core_ids=list(range(NCORES)))
    return _gather_outputs(res.results)


# revision 8
# speedup vs baseline: 1.4597x; 1.4597x over previous
"""Trainium2 Bass kernel for nn_AddDropMRR (add-drop microring resonator).

Math: for real inputs x (input_signal) and a (add_signal), the reference's
complex pipeline reduces to two magnitude outputs expressible with
per-wavelength REAL coefficient vectors (prologue-computed on device):

  e1 = x^2, e2 = (s*a)^2, e3 = x*a
  R2 = |ring_with_add|^2 = m1*e1 + m2*e3 + e2
  W2 = |through|^2       = n1*e1 + n2*e3 + n3*R2
  through = sqrt(W2);  drop = sqrt(k2c^2 * R2)

Sharding: wavelength dim (8192) split across 8 cores (1024 each). Shards are
host-transposed so wavelength lies on SBUF partitions; per-wavelength
coefficients become [128,1] per-partition scalars feeding fused
scalar_tensor_tensor ops on the vector engine, squares/sqrts on the scalar
engine. All heavy traffic (2 in + 2 out tiles of [128, 2048] f32 per chunk)
is contiguous 1 MiB DMA.
"""
import numpy as np

B = 2048          # batch
W = 8192          # wavelengths
NCORES = 8
WSH = W // NCORES  # 1024 wavelengths per core
P = 128            # SBUF partitions
NCHUNK = WSH // P  # 8 chunks per core
N_EFF = 2.4
CIRC = 2.0 * np.pi * 1e-05
KC = 2.0 * np.pi * N_EFF * CIRC
TWO_PI = float(2.0 * np.pi)


def _host_scalars(coupling_1, coupling_2, phi_1, phi_2, phi_ring, alpha):
    c1 = float(np.asarray(coupling_1).reshape(-1)[0])
    c2 = float(np.asarray(coupling_2).reshape(-1)[0])
    p1 = float(np.asarray(phi_1).reshape(-1)[0])
    pr = float(np.asarray(phi_ring).reshape(-1)[0])
    al = float(np.asarray(alpha).reshape(-1)[0])
    k1c = float(np.clip(c1, 0.01, 0.99))
    k2c = float(np.clip(c2, 0.01, 0.99))
    t1 = float(np.sqrt(1.0 - k1c * k1c))
    t2 = float(np.sqrt(1.0 - k2c * k2c))
    s = float(np.sqrt(c2))      # unclamped, as in reference
    s1 = float(np.sqrt(c1))     # unclamped
    kappa = float(al * np.sqrt(1.0 - c1 * c1) * np.sqrt(1.0 - c2 * c2))  # unclamped
    return dict(
        k1c=k1c, k2c=k2c, t1=t1, t2=t2, s=s, s1=s1, kappa=kappa, alpha=al,
        phi1=p1, phiring=pr,
        m1=(k1c * al) ** 2,
        t2s1=t2 * s1,
    )


def _build_graph(sc, loop_n=1, nchunk=NCHUNK, main_bufs=6, tmp_bufs=3, mode="full"):
    """Build the SPMD per-core graph. loop_n>1 wraps everything in a For_i
    hardware loop (identical body each iteration) for steady-state timing.
    mode: full | dma | act | dve | prologue (non-full modes are for
    engine-isolation timing probes only; outputs are garbage)."""
    import concourse.tile as tile
    from concourse import bacc, mybir

    f32 = mybir.dt.float32
    i32 = mybir.dt.int32
    AF = mybir.ActivationFunctionType
    ALU = mybir.AluOpType

    wsh = nchunk * P
    nc = bacc.Bacc("TRN2", target_bir_lowering=False, debug=False,
                   num_devices=NCORES)
    x_ext = nc.declare_dram_parameter("x_t", [wsh, B], f32, isOutput=False)
    a_ext = nc.declare_dram_parameter("a_t", [wsh, B], f32, isOutput=False)
    wl_ext = nc.declare_dram_parameter("wl_t", [P, nchunk], f32, isOutput=False)
    o1_ext = nc.declare_dram_parameter("o1_t", [wsh, B], f32, isOutput=True)
    o2_ext = nc.declare_dram_parameter("o2_t", [wsh, B], f32, isOutput=True)

    with tile.TileContext(nc) as tc:
        with tc.tile_pool(name="cst", bufs=1) as cst, \
             tc.tile_pool(name="mio", bufs=main_bufs) as mio, \
             tc.tile_pool(name="mtmp", bufs=tmp_bufs) as mtmp:

            def body(_iv=None):
                # ---------------- prologue: per-wavelength coefficients -----
                _tag = [0]

                def t(shape=(P, nchunk), dt=f32):
                    _tag[0] += 1
                    return cst.tile(list(shape), dt, tag=f"cst{_tag[0]}", name=f"cst{_tag[0]}")

                wlt = t()
                nc.sync.dma_start(wlt[:], wl_ext[:])
                inv = t()
                nc.vector.reciprocal(inv[:], wlt[:])

                # two range-reduced angles: phi and phi + pi/2
                trig = []
                for bias in (sc["phiring"], sc["phiring"] + np.pi / 2):
                    ang = t()
                    nc.vector.tensor_scalar(ang[:], inv[:], KC, float(bias),
                                            ALU.mult, ALU.add)
                    u = t()
                    nc.vector.tensor_scalar(u[:], ang[:], 1.0 / TWO_PI, None,
                                            ALU.mult)
                    ki = t(dt=i32)
                    nc.vector.tensor_copy(ki[:], u[:])
                    kf = t()
                    nc.vector.tensor_copy(kf[:], ki[:])
                    angm = t()
                    nc.vector.scalar_tensor_tensor(
                        out=angm[:], in0=kf[:], scalar=-TWO_PI, in1=ang[:],
                        op0=ALU.mult, op1=ALU.add)
                    sn = t()
                    nc.scalar.activation(sn[:], angm[:], AF.Sin)
                    trig.append(sn)
                sin_phi, cos_phi = trig

                # rotate by phi1 using host sin/cos (phi1 is a scalar input)
                c1h = float(np.cos(sc["phi1"]))
                s1h = float(np.sin(sc["phi1"]))
                tmp = t()
                nc.vector.tensor_scalar(tmp[:], cos_phi[:], s1h, None, ALU.mult)
                sin_p1 = t()
                nc.vector.scalar_tensor_tensor(
                    out=sin_p1[:], in0=sin_phi[:], scalar=c1h, in1=tmp[:],
                    op0=ALU.mult, op1=ALU.add)
                tmp2 = t()
                nc.vector.tensor_scalar(tmp2[:], sin_phi[:], s1h, None, ALU.mult)
                cos_p1 = t()
                nc.vector.scalar_tensor_tensor(
                    out=cos_p1[:], in0=cos_phi[:], scalar=c1h, in1=tmp2[:],
                    op0=ALU.mult, op1=ALU.subtract)

                ka = sc["k1c"] * sc["alpha"]
                Pv = t()
                nc.vector.tensor_scalar(Pv[:], sin_p1[:], -ka, None, ALU.mult)
                Qv = t()
                nc.vector.tensor_scalar(Qv[:], cos_p1[:], ka, None, ALU.mult)

                den_re = t()
                nc.vector.tensor_scalar(den_re[:], cos_phi[:], -sc["kappa"], 1.0,
                                        ALU.mult, ALU.add)
                d2 = t()
                nc.vector.tensor_mul(d2[:], den_re[:], den_re[:])
                s2q = t()
                nc.vector.tensor_mul(s2q[:], sin_phi[:], sin_phi[:])
                den2 = t()
                nc.vector.scalar_tensor_tensor(
                    out=den2[:], in0=s2q[:], scalar=sc["kappa"] ** 2, in1=d2[:],
                    op0=ALU.mult, op1=ALU.add)
                invd = t()
                nc.vector.reciprocal(invd[:], den2[:])

                g0 = t()
                nc.vector.tensor_mul(g0[:], den_re[:], invd[:])
                Gre = t()
                nc.vector.tensor_scalar(Gre[:], g0[:], sc["t2s1"], None, ALU.mult)
                g1 = t()
                nc.vector.tensor_mul(g1[:], sin_phi[:], invd[:])
                Gim = t()
                nc.vector.tensor_scalar(Gim[:], g1[:], sc["t2s1"] * sc["kappa"],
                                        None, ALU.mult)

                m2v = t()
                nc.vector.tensor_scalar(m2v[:], Pv[:], 2.0 * sc["s"], None, ALU.mult)
                n3v = t()
                nc.vector.tensor_scalar(n3v[:], invd[:], sc["t2s1"] ** 2, None,
                                        ALU.mult)
                z1 = t()
                nc.vector.tensor_mul(z1[:], Gre[:], Pv[:])
                z2 = t()
                nc.vector.tensor_mul(z2[:], Gim[:], Qv[:])
                z3 = t()
                nc.vector.tensor_sub(z3[:], z1[:], z2[:])
                n1v = t()
                nc.vector.tensor_scalar(n1v[:], z3[:], 2.0 * sc["t1"],
                                        sc["t1"] ** 2, ALU.mult, ALU.add)
                n2v = t()
                nc.vector.tensor_scalar(n2v[:], Gre[:], 2.0 * sc["t1"] * sc["s"],
                                        None, ALU.mult)

                # ---------------- main loop over wavelength chunks ----------
                if mode == "prologue":
                    # touch vectors so they aren't dead: copy to o1 corner
                    dummy = mio.tile([P, nchunk], f32, tag="dummy", name="dummy")
                    nc.vector.tensor_add(dummy[:], m2v[:], n1v[:])
                    nc.sync.dma_start(o1_ext[0:P, 0:nchunk], dummy[:])
                    return
                if mode == "full":
                    for c in range(nchunk):
                        rs = slice(c * P, (c + 1) * P)
                        xt = mio.tile([P, B], f32, tag="xt", name="xt")
                        nc.sync.dma_start(xt[:], x_ext[rs, :])
                        at = mio.tile([P, B], f32, tag="at", name="at")
                        nc.sync.dma_start(at[:], a_ext[rs, :])
                        e1 = mio.tile([P, B], f32, tag="e1", name="e1")
                        nc.scalar.activation(e1[:], xt[:], AF.Square)
                        e3 = mio.tile([P, B], f32, tag="e3", name="e3")
                        nc.vector.tensor_mul(e3[:], xt[:], at[:])
                        # square a in place (raw a consumed by e3)
                        nc.scalar.activation(at[:], at[:], AF.Square,
                                             scale=sc["s"])
                        # R2 accumulates in place in at
                        nc.vector.scalar_tensor_tensor(
                            out=at[:], in0=e1[:], scalar=sc["m1"], in1=at[:],
                            op0=ALU.mult, op1=ALU.add)
                        nc.vector.scalar_tensor_tensor(
                            out=at[:], in0=e3[:], scalar=m2v[:, c:c + 1],
                            in1=at[:], op0=ALU.mult, op1=ALU.add)
                        # W2 accumulates in place in xt (raw x consumed)
                        nc.scalar.activation(xt[:], e1[:], AF.Copy,
                                             scale=n1v[:, c:c + 1])
                        nc.vector.scalar_tensor_tensor(
                            out=xt[:], in0=e3[:], scalar=n2v[:, c:c + 1],
                            in1=xt[:], op0=ALU.mult, op1=ALU.add)
                        nc.vector.scalar_tensor_tensor(
                            out=xt[:], in0=at[:], scalar=n3v[:, c:c + 1],
                            in1=xt[:], op0=ALU.mult, op1=ALU.add)
                        # in-place sqrts (R2 consumed just above)
                        nc.scalar.activation(at[:], at[:], AF.Sqrt,
                                             scale=sc["k2c"] ** 2)
                        nc.scalar.activation(xt[:], xt[:], AF.Sqrt)
                        nc.sync.dma_start(o1_ext[rs, :], xt[:])
                        nc.sync.dma_start(o2_ext[rs, :], at[:])
                    return
                for c in range(nchunk):
                    rs = slice(c * P, (c + 1) * P)
                    xt = mio.tile([P, B], f32, tag="xt", name="xt")
                    nc.sync.dma_start(xt[:], x_ext[rs, :])
                    at = mio.tile([P, B], f32, tag="at", name="at")
                    nc.sync.dma_start(at[:], a_ext[rs, :])
                    if mode == "dma":
                        nc.sync.dma_start(o1_ext[rs, :], xt[:])
                        nc.sync.dma_start(o2_ext[rs, :], at[:])
                        continue
                    if mode == "act":
                        e1 = mtmp.tile([P, B], f32, tag="e1", name="e1")
                        nc.scalar.activation(e1[:], xt[:], AF.Square)
                        e2 = mtmp.tile([P, B], f32, tag="e2", name="e2")
                        nc.scalar.activation(e2[:], at[:], AF.Square, scale=sc["s"])
                        o2t = mio.tile([P, B], f32, tag="o2t", name="o2t")
                        nc.scalar.activation(o2t[:], e2[:], AF.Sqrt,
                                             scale=sc["k2c"] ** 2)
                        w0 = mtmp.tile([P, B], f32, tag="w0", name="w0")
                        nc.scalar.activation(w0[:], e1[:], AF.Copy,
                                             scale=n1v[:, c:c + 1])
                        o1t = mio.tile([P, B], f32, tag="o1t", name="o1t")
                        nc.scalar.activation(o1t[:], w0[:], AF.Sqrt)
                        nc.sync.dma_start(o1_ext[rs, :], o1t[:])
                        nc.sync.dma_start(o2_ext[rs, :], o2t[:])
                        continue
                    if mode == "dve":
                        e3 = mtmp.tile([P, B], f32, tag="e3", name="e3")
                        nc.vector.tensor_mul(e3[:], xt[:], at[:])
                        e2 = mtmp.tile([P, B], f32, tag="e2", name="e2")
                        nc.vector.scalar_tensor_tensor(
                            out=e2[:], in0=e3[:], scalar=sc["m1"], in1=xt[:],
                            op0=ALU.mult, op1=ALU.add)
                        nc.vector.scalar_tensor_tensor(
                            out=e2[:], in0=e3[:], scalar=m2v[:, c:c + 1], in1=e2[:],
                            op0=ALU.mult, op1=ALU.add)
                        w0 = mtmp.tile([P, B], f32, tag="w0", name="w0")
                        nc.vector.scalar_tensor_tensor(
                            out=w0[:], in0=e3[:], scalar=n2v[:, c:c + 1], in1=e2[:],
                            op0=ALU.mult, op1=ALU.add)
                        nc.vector.scalar_tensor_tensor(
                            out=w0[:], in0=e2[:], scalar=n3v[:, c:c + 1], in1=w0[:],
                            op0=ALU.mult, op1=ALU.add)
                        nc.sync.dma_start(o1_ext[rs, :], w0[:])
                        nc.sync.dma_start(o2_ext[rs, :], e2[:])
                        continue


            if loop_n > 1:
                with tc.For_i(0, loop_n, 1):
                    body()
            else:
                body()

    nc.compile()
    return nc


def _shard_inputs(input_signal, add_signal, wavelengths):
    x = np.ascontiguousarray(np.asarray(input_signal, dtype=np.float32))
    a = np.ascontiguousarray(np.asarray(add_signal, dtype=np.float32))
    wl = np.ascontiguousarray(np.asarray(wavelengths, dtype=np.float32))
    in_maps = []
    for i in range(NCORES):
        sl = slice(i * WSH, (i + 1) * WSH)
        in_maps.append({
            "x_t": np.ascontiguousarray(x[:, sl].T),
            "a_t": np.ascontiguousarray(a[:, sl].T),
            "wl_t": np.ascontiguousarray(wl[sl].reshape(NCHUNK, P).T),
        })
    return in_maps


def _gather_outputs(results):
    through = np.empty((B, W), np.float32)
    drop = np.empty((B, W), np.float32)
    for i in range(NCORES):
        sl = slice(i * WSH, (i + 1) * WSH)
        through[:, sl] = results[i]["o1_t"].T
        drop[:, sl] = results[i]["o2_t"].T
    return through, drop


def kernel(input_signal, add_signal, wavelengths, coupling_1, coupling_2,
           phi_1, phi_2, phi_ring, alpha):
    from concourse.bass_utils import run_bass_kernel_spmd

    sc = _host_scalars(coupling_1, coupling_2, phi_1, phi_2, phi_ring, alpha)
    nc = _build_graph(sc)
    in_maps = _shard_inputs(input_signal, add_signal, wavelengths)
    res = run_bass_kernel_spmd(nc, in_maps, core_ids=list(range(NCORES)))
    return _gather_outputs(res.results)
